# revision 7
# baseline (speedup 1.0000x reference)
"""Multi-head self-attention (B=2, S=2048, D=1024, H=16) on 8 TRN2 NeuronCores.

Sharding: data-parallel over batch (2) x tensor-parallel over head-groups (4).
Core c = b*4 + hg handles batch b, heads hg*4..hg*4+3 (4 heads, 256 features).

Per-core device program (SPMD, identical on all cores):
  - QKV projections for the core's 256 output features (column-parallel)
  - full S x S attention for its 4 heads (softmax without max-subtraction,
    denominators via an appended ones-column in the PV matmul)
  - partial output projection (row-parallel): out_partial^T [1024, 2048]
Host: shards/transposes inputs, sums the 4 partial outputs per batch
(the "all-reduce"), adds bo, and untransposes.

All matmuls run in float32r (TF32-like, ~11 mantissa bits, 1 cycle/row on the
PE vs 4 for plain fp32); accumulation is fp32 in PSUM.
"""

import numpy as np

B, S, D = 2, 2048, 1024
H, DK = 16, 64
NCORES = 8
HG = 4          # head groups (tensor parallel)
HPG = 4         # heads per group
F = HPG * DK    # 256 local features per core
SCALE = 1.0 / np.sqrt(DK)

_compiled = {}


def _build():
    import concourse.bacc as bacc
    import concourse.tile as tile
    from concourse import mybir

    f32 = mybir.dt.float32
    f32r = mybir.dt.float32r
    Exp = mybir.ActivationFunctionType.Exp
    mult = mybir.AluOpType.mult

    nc = bacc.Bacc("TRN2", target_bir_lowering=False, debug=False,
                   enable_asserts=True, num_devices=NCORES)

    xq = nc.dram_tensor("xq", (D, S), f32r, kind="ExternalInput")   # q[b].T
    xk = nc.dram_tensor("xk", (D, S), f32r, kind="ExternalInput")
    xv = nc.dram_tensor("xv", (D, S), f32r, kind="ExternalInput")
    wq = nc.dram_tensor("wq", (D, F), f32r, kind="ExternalInput")   # Wq[rows].T
    wk = nc.dram_tensor("wk", (D, F), f32r, kind="ExternalInput")
    wv = nc.dram_tensor("wv", (D, F), f32r, kind="ExternalInput")
    wo = nc.dram_tensor("wo", (F, D), f32r, kind="ExternalInput")   # Wo[:, cols].T
    bq = nc.dram_tensor("bq", (128, 2), f32, kind="ExternalInput")  # bias, f-tiled
    bk = nc.dram_tensor("bk", (128, 2), f32, kind="ExternalInput")
    bv = nc.dram_tensor("bv", (1, F), f32, kind="ExternalInput")
    out = nc.dram_tensor("out", (D, S), f32, kind="ExternalOutput")  # partial^T

    NDT = D // 128   # 8 d-tiles
    NST = S // 128   # 16 s-tiles (j tiles)
    NSB = S // 512   # 4 s-blocks (i blocks)

    with tile.TileContext(nc) as tc:
        import contextlib
        with contextlib.ExitStack() as ctx:
            consts = ctx.enter_context(tc.tile_pool(name="consts", bufs=1))
            big = ctx.enter_context(tc.tile_pool(name="big", bufs=10))
            acts = ctx.enter_context(tc.tile_pool(name="acts", bufs=1))
            ostage = ctx.enter_context(tc.tile_pool(name="ostage", bufs=3))
            small = ctx.enter_context(tc.tile_pool(name="small", bufs=2))
            ps = ctx.enter_context(tc.tile_pool(name="ps", bufs=4, space="PSUM"))

            # ---- constants ----
            wq_sb = consts.tile([128, NDT, F], f32r, tag="wq")
            wk_sb = consts.tile([128, NDT, F], f32r, tag="wk")
            wv_sb = consts.tile([128, NDT, F], f32r, tag="wv")
            nc.sync.dma_start(wq_sb[:], wq.ap().rearrange("(t p) f -> p t f", p=128))
            nc.sync.dma_start(wk_sb[:], wk.ap().rearrange("(t p) f -> p t f", p=128))
            nc.sync.dma_start(wv_sb[:], wv.ap().rearrange("(t p) f -> p t f", p=128))
            wo_sb = consts.tile([128, 2, D], f32r, tag="wo")
            nc.sync.dma_start(wo_sb[:], wo.ap().rearrange("(t p) e -> p t e", p=128))
            bq_sb = consts.tile([128, 2], f32, tag="bq")
            bk_sb = consts.tile([128, 2], f32, tag="bk")
            nc.sync.dma_start(bq_sb[:], bq.ap()[:])
            nc.sync.dma_start(bk_sb[:], bk.ap()[:])
            bv_sb = consts.tile([128, F], f32, tag="bv")
            nc.sync.dma_start(bv_sb[:], bv.ap().to_broadcast((128, F)))

            # persistent activations
            # qh/kh: [f, s] transposed projections, per (ft, sb) tiles
            qh_t = [[acts.tile([128, 512], f32r, tag=f"qh{ft}{sb}", name=f"qh{ft}{sb}")
                     for sb in range(NSB)] for ft in range(2)]
            kh_t = [[acts.tile([128, 512], f32r, tag=f"kh{ft}{sb}", name=f"kh{ft}{sb}")
                     for sb in range(NSB)] for ft in range(2)]
            # vh: [s, h, c+1] with ones column at c=64 (PV denominator trick)
            vh_t = [acts.tile([128, HPG, DK + 1], f32r, tag=f"vh{st}", name=f"vh{st}")
                    for st in range(NST)]
            for st in range(NST):
                nc.vector.memset(vh_t[st][:, :, DK:DK + 1].bitcast(f32), 1.0)
            # y: normalized attention output, [f, s] per (ft, sb)
            y_t = [[acts.tile([128, 512], f32r, tag=f"y{ft}{sb}", name=f"y{ft}{sb}")
                    for sb in range(NSB)] for ft in range(2)]

            # ---- Q / K projections: out[f, s] = W.T-shard @ x.T ----
            for name, xdram, w_sb, b_sb, dst in (
                ("q", xq, wq_sb, bq_sb, qh_t),
                ("k", xk, wk_sb, bk_sb, kh_t),
            ):
                pst = [ps.tile([128, 2, 512], f32, tag="mm", name=f"ps_{name}{i}") for i in range(4)]
                for dt in range(NDT):
                    x_tile = big.tile([128, S], f32r, tag="big")
                    nc.sync.dma_start(x_tile[:], xdram.ap()[dt * 128:(dt + 1) * 128, :])
                    for ft in range(2):
                        for sb in range(NSB):
                            nc.tensor.matmul(
                                pst[2 * ft + sb // 2][:, sb % 2, :],
                                w_sb[:, dt, ft * 128:(ft + 1) * 128],
                                x_tile[:, sb * 512:(sb + 1) * 512],
                                start=(dt == 0), stop=(dt == NDT - 1),
                            )
                for ft in range(2):
                    for sb in range(NSB):
                        nc.vector.tensor_scalar_add(
                            dst[ft][sb][:],
                            pst[2 * ft + sb // 2][:, sb % 2, :],
                            b_sb[:, ft:ft + 1],
                        )

            # ---- V projection: vh[s, f] = x @ Wv-shard.T (natural layout) ----
            xvt = []
            for dt in range(NDT):
                x_tile = big.tile([128, S], f32r, tag="big")
                nc.sync.dma_start(x_tile[:], xv.ap()[dt * 128:(dt + 1) * 128, :])
                xvt.append(x_tile)
            for st in range(NST):
                pv = ps.tile([128, 2, 512], f32, tag="mm")
                for dt in range(NDT):
                    nc.tensor.matmul(
                        pv[:, 0, 0:F],
                        xvt[dt][:, st * 128:(st + 1) * 128],
                        wv_sb[:, dt, :],
                        start=(dt == 0), stop=(dt == NDT - 1),
                    )
                nc.vector.tensor_tensor(
                    vh_t[st][:, :, 0:DK],
                    pv[:, 0, 0:F].rearrange("p (h c) -> p h c", h=HPG),
                    bv_sb[:].rearrange("p (h c) -> p h c", h=HPG),
                    mybir.AluOpType.add,
                )

            # ---- attention, head pairs (row-packed QK), + PV with ones col ----
            for pr in range(2):            # head pair: heads (2pr, 2pr+1)
                ft = pr                    # feature tile holding this pair
                for ib in range(NSB):      # query block, 512 wide
                    pv_ps = [ps.tile([128, 2, 512], f32, tag="mm", name=f"pv{pr}{ib}_{i}") for i in range(2)]
                    at_tiles = []
                    for jc in range(NST // 4):   # 4-jt attn tiles
                        at = [big.tile([128, 4, 512], f32r, tag="big", name=f"at{pr}{ib}{jc}_{i}") for i in range(2)]
                        at_tiles.append(at)
                        for half in range(2):    # 2-jt score chunks
                            sc = [None, None]
                            for hh in range(2):  # head in pair
                                base = hh * 64
                                sc[hh] = ps.tile([128, 2, 512], f32, tag="mm", name=f"sc{hh}")
                                for jj in range(2):
                                    jt = jc * 4 + half * 2 + jj
                                    nc.tensor.matmul(
                                        sc[hh][:, jj, :],
                                        kh_t[ft][jt // 4][base:base + 64,
                                                          (jt % 4) * 128:(jt % 4 + 1) * 128],
                                        qh_t[ft][ib][base:base + 64, :],
                                        start=True, stop=True,
                                        tile_position=(base, 0),
                                    )
                            for hh in range(2):
                                nc.scalar.activation(
                                    at[hh][:, half * 2:half * 2 + 2, :],
                                    sc[hh][:, :, :],
                                    Exp, scale=float(SCALE),
                                )
                    # PV: accumulate over all 16 jt; lhsT = [vh | ones] (65 cols)
                    for hh in range(2):
                        h = 2 * pr + hh
                        for jt in range(NST):
                            nc.tensor.matmul(
                                pv_ps[hh][0:DK + 1, 0, :],
                                vh_t[jt][:, h, :],
                                at_tiles[jt // 4][hh][:, jt % 4, :],
                                start=(jt == 0), stop=(jt == NST - 1),
                            )
                        # normalize: y = pv[0:64] * (1 / pv[64])
                        rec = small.tile([1, 512], f32, tag="rec")
                        nc.vector.reciprocal(rec[:], pv_ps[hh][DK:DK + 1, 0, :])
                        rb = small.tile([64, 512], f32, tag="rb")
                        nc.gpsimd.partition_broadcast(rb[:], rec[:])
                        nc.vector.tensor_tensor(
                            y_t[ft][ib][hh * 64:hh * 64 + 64, :],
                            pv_ps[hh][0:DK, 0, :],
                            rb[:],
                            mult,
                        )

            # ---- output projection: out^T[e, s] = Wo-shard.T @ y ----
            for et in range(NDT):
                for sbh in range(2):
                    po = ps.tile([128, 2, 512], f32, tag="mm")
                    for j in range(2):
                        sb = sbh * 2 + j
                        for ft in range(2):
                            nc.tensor.matmul(
                                po[:, j, :],
                                wo_sb[:, ft, et * 128:(et + 1) * 128],
                                y_t[ft][sb][:],
                                start=(ft == 0), stop=(ft == 1),
                            )
                    o_sb = ostage.tile([128, 2, 512], f32, tag="ost")
                    nc.vector.tensor_copy(o_sb[:], po[:])
                    nc.sync.dma_start(
                        out.ap()[et * 128:(et + 1) * 128, sbh * 1024:(sbh + 1) * 1024],
                        o_sb[:].rearrange("p a b -> p (a b)"),
                    )

    nc.compile()
    return nc


def _get_nc():
    if "nc" not in _compiled:
        _compiled["nc"] = _build()
    return _compiled["nc"]


def kernel(q, k, v, Wq, bq, Wk, bk, Wv, bv, Wo, bo):
    outp, _ = _run(q, k, v, Wq, bq, Wk, bk, Wv, bv, Wo, bo)
    return outp


def _run(q, k, v, Wq, bq, Wk, bk, Wv, bv, Wo, bo, **run_kwargs):
    from concourse.bass_utils import run_bass_kernel_spmd

    nc = _get_nc()

    q = np.asarray(q, np.float32)
    k = np.asarray(k, np.float32)
    v = np.asarray(v, np.float32)
    Wq = np.asarray(Wq, np.float32)
    Wk = np.asarray(Wk, np.float32)
    Wv = np.asarray(Wv, np.float32)
    Wo = np.asarray(Wo, np.float32)
    bq = np.asarray(bq, np.float32)
    bk = np.asarray(bk, np.float32)
    bv = np.asarray(bv, np.float32)
    bo = np.asarray(bo, np.float32)

    xqT = [np.ascontiguousarray(q[b].T) for b in range(B)]
    xkT = [np.ascontiguousarray(k[b].T) for b in range(B)]
    xvT = [np.ascontiguousarray(v[b].T) for b in range(B)]

    in_maps = []
    for c in range(NCORES):
        b, hg = divmod(c, HG)
        rows = slice(hg * F, (hg + 1) * F)
        in_maps.append({
            "xq": xqT[b], "xk": xkT[b], "xv": xvT[b],
            "wq": np.ascontiguousarray(Wq[rows].T),
            "wk": np.ascontiguousarray(Wk[rows].T),
            "wv": np.ascontiguousarray(Wv[rows].T),
            "wo": np.ascontiguousarray(Wo[:, rows].T),
            "bq": np.ascontiguousarray(bq[rows].reshape(2, 128).T),
            "bk": np.ascontiguousarray(bk[rows].reshape(2, 128).T),
            "bv": np.ascontiguousarray(bv[rows].reshape(1, F)),
        })

    res = run_bass_kernel_spmd(nc, in_maps, core_ids=list(range(NCORES)), **run_kwargs)

    outp = np.empty((B, S, D), np.float32)
    for b in range(B):
        acc = res.results[b * HG]["out"].astype(np.float32)
        for hg in range(1, HG):
            acc = acc + res.results[b * HG + hg]["out"]
        outp[b] = acc.T + bo[None, :]
    return outp, res


# revision 8
# speedup vs baseline: 1.1461x; 1.1461x over previous
"""Multi-head self-attention (B=2, S=2048, D=1024, H=16) on 8 TRN2 NeuronCores.

Sharding: data-parallel over batch (2) x tensor-parallel over head-groups (4).
Core c = b*4 + hg handles batch b, heads hg*4..hg*4+3 (4 heads, 256 features).

Per-core device program (SPMD, identical on all cores):
  - QKV projections for the core's 256 output features (column-parallel)
  - full S x S attention for its 4 heads (softmax without max-subtraction,
    denominators via an appended ones-column in the PV matmul)
  - partial output projection (row-parallel): out_partial^T [1024, 2048]
Host: shards/transposes inputs, sums the 4 partial outputs per batch
(the "all-reduce"), adds bo, and untransposes.

All matmuls run in float32r (TF32-like, ~11 mantissa bits, 1 cycle/row on the
PE vs 4 for plain fp32); accumulation is fp32 in PSUM.
"""

import numpy as np

B, S, D = 2, 2048, 1024
H, DK = 16, 64
NCORES = 8
HG = 4          # head groups (tensor parallel)
HPG = 4         # heads per group
F = HPG * DK    # 256 local features per core
SCALE = 1.0 / np.sqrt(DK)

_compiled = {}


def _build():
    import concourse.bacc as bacc
    import concourse.tile as tile
    from concourse import mybir

    f32 = mybir.dt.float32
    bf16 = mybir.dt.bfloat16
    Exp = mybir.ActivationFunctionType.Exp
    mult = mybir.AluOpType.mult

    nc = bacc.Bacc("TRN2", target_bir_lowering=False, debug=False,
                   enable_asserts=True, num_devices=NCORES)

    xq = nc.dram_tensor("xq", (D, S), bf16, kind="ExternalInput")   # q[b].T
    xk = nc.dram_tensor("xk", (D, S), bf16, kind="ExternalInput")
    xv = nc.dram_tensor("xv", (D, S), bf16, kind="ExternalInput")
    wq = nc.dram_tensor("wq", (D, F), bf16, kind="ExternalInput")   # Wq[rows].T
    wk = nc.dram_tensor("wk", (D, F), bf16, kind="ExternalInput")
    wv = nc.dram_tensor("wv", (D, F), bf16, kind="ExternalInput")
    wo = nc.dram_tensor("wo", (F, D), bf16, kind="ExternalInput")   # Wo[:, cols].T
    bq = nc.dram_tensor("bq", (128, 2), f32, kind="ExternalInput")  # bias, f-tiled
    bk = nc.dram_tensor("bk", (128, 2), f32, kind="ExternalInput")
    bv = nc.dram_tensor("bv", (1, F), f32, kind="ExternalInput")
    out = nc.dram_tensor("out", (D, S), f32, kind="ExternalOutput")  # partial^T

    NDT = D // 128   # 8 d-tiles
    NST = S // 128   # 16 s-tiles (j tiles)
    NSB = S // 512   # 4 s-blocks (i blocks)

    with tile.TileContext(nc) as tc:
        import contextlib
        with contextlib.ExitStack() as ctx:
            consts = ctx.enter_context(tc.tile_pool(name="consts", bufs=1))
            big = ctx.enter_context(tc.tile_pool(name="big", bufs=16))
            acts = ctx.enter_context(tc.tile_pool(name="acts", bufs=1))
            ostage = ctx.enter_context(tc.tile_pool(name="ostage", bufs=3))
            small = ctx.enter_context(tc.tile_pool(name="small", bufs=2))
            ps = ctx.enter_context(tc.tile_pool(name="ps", bufs=4, space="PSUM"))

            # ---- constants ----
            wq_sb = consts.tile([128, NDT, F], bf16, tag="wq")
            wk_sb = consts.tile([128, NDT, F], bf16, tag="wk")
            wv_sb = consts.tile([128, NDT, F], bf16, tag="wv")
            nc.sync.dma_start(wq_sb[:], wq.ap().rearrange("(t p) f -> p t f", p=128))
            nc.sync.dma_start(wk_sb[:], wk.ap().rearrange("(t p) f -> p t f", p=128))
            nc.sync.dma_start(wv_sb[:], wv.ap().rearrange("(t p) f -> p t f", p=128))
            wo_sb = consts.tile([128, 2, D], bf16, tag="wo")
            nc.sync.dma_start(wo_sb[:], wo.ap().rearrange("(t p) e -> p t e", p=128))
            bq_sb = consts.tile([128, 2], f32, tag="bq")
            bk_sb = consts.tile([128, 2], f32, tag="bk")
            nc.sync.dma_start(bq_sb[:], bq.ap()[:])
            nc.sync.dma_start(bk_sb[:], bk.ap()[:])
            bv_sb = consts.tile([128, F], f32, tag="bv")
            nc.sync.dma_start(bv_sb[:], bv.ap().to_broadcast((128, F)))

            # persistent activations
            # qh/kh: [f, s] transposed projections, per (ft, sb) tiles
            qh_t = [[acts.tile([128, 512], bf16, tag=f"qh{ft}{sb}", name=f"qh{ft}{sb}")
                     for sb in range(NSB)] for ft in range(2)]
            kh_t = [[acts.tile([128, 512], bf16, tag=f"kh{ft}{sb}", name=f"kh{ft}{sb}")
                     for sb in range(NSB)] for ft in range(2)]
            # vh: [s, h, c+1] with ones column at c=64 (PV denominator trick)
            vh_t = [acts.tile([128, HPG, DK + 1], bf16, tag=f"vh{st}", name=f"vh{st}")
                    for st in range(NST)]
            for st in range(NST):
                nc.vector.memset(vh_t[st][:, :, DK:DK + 1], 1.0)
            # y: normalized attention output, [f, s] per (ft, sb)
            y_t = [[acts.tile([128, 512], bf16, tag=f"y{ft}{sb}", name=f"y{ft}{sb}")
                    for sb in range(NSB)] for ft in range(2)]

            # ---- Q / K projections: out[f, s] = W.T-shard @ x.T ----
            for name, xdram, w_sb, b_sb, dst in (
                ("q", xq, wq_sb, bq_sb, qh_t),
                ("k", xk, wk_sb, bk_sb, kh_t),
            ):
                pst = [ps.tile([128, 2, 512], f32, tag="mm", name=f"ps_{name}{i}") for i in range(4)]
                for dt in range(NDT):
                    x_tile = big.tile([128, S], bf16, tag="big")
                    nc.sync.dma_start(x_tile[:], xdram.ap()[dt * 128:(dt + 1) * 128, :])
                    for ft in range(2):
                        for sb in range(NSB):
                            nc.tensor.matmul(
                                pst[2 * ft + sb // 2][:, sb % 2, :],
                                w_sb[:, dt, ft * 128:(ft + 1) * 128],
                                x_tile[:, sb * 512:(sb + 1) * 512],
                                start=(dt == 0), stop=(dt == NDT - 1),
                            )
                for ft in range(2):
                    for sb in range(NSB):
                        nc.vector.tensor_scalar_add(
                            dst[ft][sb][:],
                            pst[2 * ft + sb // 2][:, sb % 2, :],
                            b_sb[:, ft:ft + 1],
                        )

            # ---- V projection: vh[s, f] = x @ Wv-shard.T (natural layout) ----
            xvt = []
            for dt in range(NDT):
                x_tile = big.tile([128, S], bf16, tag="big")
                nc.sync.dma_start(x_tile[:], xv.ap()[dt * 128:(dt + 1) * 128, :])
                xvt.append(x_tile)
            for st in range(NST):
                pv = ps.tile([128, 2, 512], f32, tag="mm")
                for dt in range(NDT):
                    nc.tensor.matmul(
                        pv[:, 0, 0:F],
                        xvt[dt][:, st * 128:(st + 1) * 128],
                        wv_sb[:, dt, :],
                        start=(dt == 0), stop=(dt == NDT - 1),
                    )
                nc.vector.tensor_tensor(
                    vh_t[st][:, :, 0:DK],
                    pv[:, 0, 0:F].rearrange("p (h c) -> p h c", h=HPG),
                    bv_sb[:].rearrange("p (h c) -> p h c", h=HPG),
                    mybir.AluOpType.add,
                )

            # ---- attention, head pairs (row-packed QK), + PV with ones col ----
            for pr in range(2):            # head pair: heads (2pr, 2pr+1)
                ft = pr                    # feature tile holding this pair
                for ib in range(NSB):      # query block, 512 wide
                    pv_ps = [ps.tile([128, 2, 512], f32, tag="mm", name=f"pv{pr}{ib}_{i}") for i in range(2)]
                    at_tiles = []
                    for jc in range(NST // 4):   # 4-jt attn tiles
                        at = [big.tile([128, 4, 512], bf16, tag="big", name=f"at{pr}{ib}{jc}_{i}") for i in range(2)]
                        at_tiles.append(at)
                        for half in range(2):    # 2-jt score chunks
                            sc = [None, None]
                            for hh in range(2):  # head in pair
                                base = hh * 64
                                sc[hh] = ps.tile([128, 2, 512], f32, tag="mm", name=f"sc{hh}")
                                for jj in range(2):
                                    jt = jc * 4 + half * 2 + jj
                                    nc.tensor.matmul(
                                        sc[hh][:, jj, :],
                                        kh_t[ft][jt // 4][base:base + 64,
                                                          (jt % 4) * 128:(jt % 4 + 1) * 128],
                                        qh_t[ft][ib][base:base + 64, :],
                                        start=True, stop=True,
                                        tile_position=(base, 0),
                                    )
                            for hh in range(2):
                                nc.scalar.activation(
                                    at[hh][:, half * 2:half * 2 + 2, :],
                                    sc[hh][:, :, :],
                                    Exp, scale=float(SCALE),
                                )
                    # PV: accumulate over all 16 jt; lhsT = [vh | ones] (65 cols)
                    for hh in range(2):
                        h = 2 * pr + hh
                        for jt in range(NST):
                            nc.tensor.matmul(
                                pv_ps[hh][0:DK + 1, 0, :],
                                vh_t[jt][:, h, :],
                                at_tiles[jt // 4][hh][:, jt % 4, :],
                                start=(jt == 0), stop=(jt == NST - 1),
                            )
                        # normalize: y = pv[0:64] * (1 / pv[64])
                        lnd = small.tile([1, 512], f32, tag="lnd")
                        nc.scalar.activation(lnd[:], pv_ps[hh][DK:DK + 1, 0, :],
                                             mybir.ActivationFunctionType.Ln)
                        rec = small.tile([1, 512], f32, tag="rec")
                        nc.scalar.activation(rec[:], lnd[:],
                                             mybir.ActivationFunctionType.Exp,
                                             scale=-1.0)
                        rb = small.tile([64, 512], f32, tag="rb")
                        nc.gpsimd.partition_broadcast(rb[:], rec[:])
                        nc.vector.tensor_tensor(
                            y_t[ft][ib][hh * 64:hh * 64 + 64, :],
                            pv_ps[hh][0:DK, 0, :],
                            rb[:],
                            mult,
                        )

            # ---- output projection: out^T[e, s] = Wo-shard.T @ y ----
            for et in range(NDT):
                for sbh in range(2):
                    po = ps.tile([128, 2, 512], f32, tag="mm")
                    for j in range(2):
                        sb = sbh * 2 + j
                        for ft in range(2):
                            nc.tensor.matmul(
                                po[:, j, :],
                                wo_sb[:, ft, et * 128:(et + 1) * 128],
                                y_t[ft][sb][:],
                                start=(ft == 0), stop=(ft == 1),
                            )
                    o_sb = ostage.tile([128, 2, 512], f32, tag="ost")
                    nc.vector.tensor_copy(o_sb[:], po[:])
                    nc.sync.dma_start(
                        out.ap()[et * 128:(et + 1) * 128, sbh * 1024:(sbh + 1) * 1024],
                        o_sb[:].rearrange("p a b -> p (a b)"),
                    )

    nc.compile()
    return nc


def _get_nc():
    if "nc" not in _compiled:
        _compiled["nc"] = _build()
    return _compiled["nc"]


def kernel(q, k, v, Wq, bq, Wk, bk, Wv, bv, Wo, bo):
    outp, _ = _run(q, k, v, Wq, bq, Wk, bk, Wv, bv, Wo, bo)
    return outp


def _run(q, k, v, Wq, bq, Wk, bk, Wv, bv, Wo, bo, **run_kwargs):
    from concourse.bass_utils import run_bass_kernel_spmd

    nc = _get_nc()

    q = np.asarray(q, np.float32)
    k = np.asarray(k, np.float32)
    v = np.asarray(v, np.float32)
    Wq = np.asarray(Wq, np.float32)
    Wk = np.asarray(Wk, np.float32)
    Wv = np.asarray(Wv, np.float32)
    Wo = np.asarray(Wo, np.float32)
    bq = np.asarray(bq, np.float32)
    bk = np.asarray(bk, np.float32)
    bv = np.asarray(bv, np.float32)
    bo = np.asarray(bo, np.float32)

    import ml_dtypes
    bf = ml_dtypes.bfloat16
    xqT = [np.ascontiguousarray(q[b].T).astype(bf) for b in range(B)]
    xkT = [np.ascontiguousarray(k[b].T).astype(bf) for b in range(B)]
    xvT = [np.ascontiguousarray(v[b].T).astype(bf) for b in range(B)]

    in_maps = []
    for c in range(NCORES):
        b, hg = divmod(c, HG)
        rows = slice(hg * F, (hg + 1) * F)
        in_maps.append({
            "xq": xqT[b], "xk": xkT[b], "xv": xvT[b],
            "wq": np.ascontiguousarray(Wq[rows].T).astype(bf),
            "wk": np.ascontiguousarray(Wk[rows].T).astype(bf),
            "wv": np.ascontiguousarray(Wv[rows].T).astype(bf),
            "wo": np.ascontiguousarray(Wo[:, rows].T).astype(bf),
            "bq": np.ascontiguousarray(bq[rows].reshape(2, 128).T),
            "bk": np.ascontiguousarray(bk[rows].reshape(2, 128).T),
            "bv": np.ascontiguousarray(bv[rows].reshape(1, F)),
        })

    res = run_bass_kernel_spmd(nc, in_maps, core_ids=list(range(NCORES)), **run_kwargs)

    outp = np.empty((B, S, D), np.float32)
    for b in range(B):
        acc = res.results[b * HG]["out"].astype(np.float32)
        for hg in range(1, HG):
            acc = acc + res.results[b * HG + hg]["out"]
        outp[b] = acc.T + bo[None, :]
    return outp, res


# revision 11
# speedup vs baseline: 1.1877x; 1.0363x over previous
"""Multi-head self-attention (B=2, S=2048, D=1024, H=16) on 8 TRN2 NeuronCores.

Sharding: data-parallel over batch (2) x tensor-parallel over head-groups (4).
Core c = b*4 + hg handles batch b, heads hg*4..hg*4+3 (4 heads, 256 features).

Per-core device program (SPMD, identical on all cores):
  - QKV projections for the core's 256 output features (column-parallel)
  - full S x S attention for its 4 heads (softmax without max-subtraction,
    denominators via an appended ones-column in the PV matmul)
  - partial output projection (row-parallel): out_partial^T [1024, 2048]
Host: shards/transposes inputs, sums the 4 partial outputs per batch
(the "all-reduce"), adds bo, and untransposes.

All matmuls run in float32r (TF32-like, ~11 mantissa bits, 1 cycle/row on the
PE vs 4 for plain fp32); accumulation is fp32 in PSUM.
"""

import numpy as np

B, S, D = 2, 2048, 1024
H, DK = 16, 64
NCORES = 8
HG = 4          # head groups (tensor parallel)
HPG = 4         # heads per group
F = HPG * DK    # 256 local features per core
SCALE = 1.0 / np.sqrt(DK)

_compiled = {}


def _build():
    import concourse.bacc as bacc
    import concourse.tile as tile
    from concourse import mybir

    f32 = mybir.dt.float32
    bf16 = mybir.dt.bfloat16
    Exp = mybir.ActivationFunctionType.Exp
    mult = mybir.AluOpType.mult

    nc = bacc.Bacc("TRN2", target_bir_lowering=False, debug=False,
                   enable_asserts=True, num_devices=NCORES)

    xq = nc.dram_tensor("xq", (D, S), bf16, kind="ExternalInput")   # q[b].T
    xk = nc.dram_tensor("xk", (D, S), bf16, kind="ExternalInput")
    xv = nc.dram_tensor("xv", (D, S), bf16, kind="ExternalInput")
    wq = nc.dram_tensor("wq", (D, F), bf16, kind="ExternalInput")   # Wq[rows].T
    wk = nc.dram_tensor("wk", (D, F), bf16, kind="ExternalInput")
    wv = nc.dram_tensor("wv", (D, F), bf16, kind="ExternalInput")
    wo = nc.dram_tensor("wo", (F, D), bf16, kind="ExternalInput")   # Wo[:, cols].T
    bq = nc.dram_tensor("bq", (128, 2), f32, kind="ExternalInput")  # bias, f-tiled
    bk = nc.dram_tensor("bk", (128, 2), f32, kind="ExternalInput")
    bv = nc.dram_tensor("bv", (1, F), f32, kind="ExternalInput")
    out = nc.dram_tensor("out", (D, S), f32, kind="ExternalOutput")  # partial^T

    NDT = D // 128   # 8 d-tiles
    NST = S // 128   # 16 s-tiles (j tiles)
    NSB = S // 512   # 4 s-blocks (i blocks)

    with tile.TileContext(nc) as tc:
        import contextlib
        with contextlib.ExitStack() as ctx:
            consts = ctx.enter_context(tc.tile_pool(name="consts", bufs=1))
            big = ctx.enter_context(tc.tile_pool(name="big", bufs=24))
            acts = ctx.enter_context(tc.tile_pool(name="acts", bufs=1))
            ostage = ctx.enter_context(tc.tile_pool(name="ostage", bufs=3))
            small = ctx.enter_context(tc.tile_pool(name="small", bufs=2))
            ps = ctx.enter_context(tc.tile_pool(name="ps", bufs=1, space="PSUM"))

            # ---- constants ----
            wq_sb = consts.tile([128, NDT, F], bf16, tag="wq")
            wk_sb = consts.tile([128, NDT, F], bf16, tag="wk")
            wv_sb = consts.tile([128, NDT, F], bf16, tag="wv")
            nc.sync.dma_start(wq_sb[:], wq.ap().rearrange("(t p) f -> p t f", p=128))
            nc.sync.dma_start(wk_sb[:], wk.ap().rearrange("(t p) f -> p t f", p=128))
            nc.sync.dma_start(wv_sb[:], wv.ap().rearrange("(t p) f -> p t f", p=128))
            wo_sb = consts.tile([128, 2, D], bf16, tag="wo")
            nc.sync.dma_start(wo_sb[:], wo.ap().rearrange("(t p) e -> p t e", p=128))
            bq_sb = consts.tile([128, 2], f32, tag="bq")
            bk_sb = consts.tile([128, 2], f32, tag="bk")
            nc.sync.dma_start(bq_sb[:], bq.ap()[:])
            nc.sync.dma_start(bk_sb[:], bk.ap()[:])
            bv_sb = consts.tile([128, F], f32, tag="bv")
            nc.sync.dma_start(bv_sb[:], bv.ap().to_broadcast((128, F)))

            # persistent activations
            # qh/kh: [f, s] transposed projections, per (ft, sb) tiles
            qh_t = [[acts.tile([128, 512], bf16, tag=f"qh{ft}{sb}", name=f"qh{ft}{sb}")
                     for sb in range(NSB)] for ft in range(2)]
            kh_t = [[acts.tile([128, 512], bf16, tag=f"kh{ft}{sb}", name=f"kh{ft}{sb}")
                     for sb in range(NSB)] for ft in range(2)]
            # vh: [s, h, c+1] with ones column at c=64 (PV denominator trick)
            vh_t = [acts.tile([128, HPG, DK + 1], bf16, tag=f"vh{st}", name=f"vh{st}")
                    for st in range(NST)]
            for st in range(NST):
                nc.vector.memset(vh_t[st][:, :, DK:DK + 1], 1.0)
            # y: normalized attention output, [f, s] per (ft, sb)
            y_t = [[acts.tile([128, 512], bf16, tag=f"y{ft}{sb}", name=f"y{ft}{sb}")
                    for sb in range(NSB)] for ft in range(2)]

            # ---- Q / K projections: out[f, s] = W.T-shard @ x.T ----
            for name, xdram, w_sb, b_sb, dst in (
                ("q", xq, wq_sb, bq_sb, qh_t),
                ("k", xk, wk_sb, bk_sb, kh_t),
            ):
                xts = []
                for dt in range(NDT):
                    x_tile = big.tile([128, S], bf16, tag="big")
                    nc.sync.dma_start(x_tile[:], xdram.ap()[dt * 128:(dt + 1) * 128, :])
                    xts.append(x_tile)
                for ft in range(2):
                    pst = [ps.tile([128, 2, 512], f32, tag="sc", bufs=3, name=f"ps_{name}{ft}{i}")
                           for i in range(2)]
                    for dt in range(NDT):
                        for sb in range(NSB):
                            nc.tensor.matmul(
                                pst[sb // 2][:, sb % 2, :],
                                w_sb[:, dt, ft * 128:(ft + 1) * 128],
                                xts[dt][:, sb * 512:(sb + 1) * 512],
                                start=(dt == 0), stop=(dt == NDT - 1),
                            )
                    for sb in range(NSB):
                        nc.vector.tensor_scalar_add(
                            dst[ft][sb][:],
                            pst[sb // 2][:, sb % 2, :],
                            b_sb[:, ft:ft + 1],
                        )

            # ---- V projection: vh[s, f] = x @ Wv-shard.T (natural layout) ----
            xvt = []
            for dt in range(NDT):
                x_tile = big.tile([128, S], bf16, tag="big")
                nc.sync.dma_start(x_tile[:], xv.ap()[dt * 128:(dt + 1) * 128, :])
                xvt.append(x_tile)
            for st in range(NST):
                pv = ps.tile([128, 512], f32, tag="pv", bufs=2, name=f"vps{st}")
                for dt in range(NDT):
                    nc.tensor.matmul(
                        pv[:, 0:F],
                        xvt[dt][:, st * 128:(st + 1) * 128],
                        wv_sb[:, dt, :],
                        start=(dt == 0), stop=(dt == NDT - 1),
                    )
                nc.vector.tensor_tensor(
                    vh_t[st][:, :, 0:DK],
                    pv[:, 0:F].rearrange("p (h c) -> p h c", h=HPG),
                    bv_sb[:].rearrange("p (h c) -> p h c", h=HPG),
                    mybir.AluOpType.add,
                )

            # ---- attention, head pairs (row-packed QK), + PV with ones col ----
            for pr in range(2):            # head pair: heads (2pr, 2pr+1)
                ft = pr                    # feature tile holding this pair
                for ib in range(NSB):      # query block, 512 wide
                    pv_ps = [ps.tile([128, 512], f32, tag="pv", bufs=2, name=f"pv{pr}{ib}_{i}") for i in range(2)]
                    at_tiles = []
                    for jc in range(NST // 4):   # 4-jt attn tiles
                        at = [big.tile([128, 4, 512], bf16, tag="big", name=f"at{pr}{ib}{jc}_{i}") for i in range(2)]
                        at_tiles.append(at)
                        for half in range(2):    # 2-jt score chunks
                            sc = [None, None]
                            for hh in range(2):  # head in pair
                                base = hh * 64
                                sc[hh] = ps.tile([128, 2, 512], f32, tag="sc", bufs=3, name=f"sc{hh}")
                                for jj in range(2):
                                    jt = jc * 4 + half * 2 + jj
                                    nc.tensor.matmul(
                                        sc[hh][:, jj, :],
                                        kh_t[ft][jt // 4][base:base + 64,
                                                          (jt % 4) * 128:(jt % 4 + 1) * 128],
                                        qh_t[ft][ib][base:base + 64, :],
                                        start=True, stop=True,
                                        tile_position=(base, 0),
                                    )
                            for hh in range(2):
                                nc.scalar.activation(
                                    at[hh][:, half * 2:half * 2 + 2, :],
                                    sc[hh][:, :, :],
                                    Exp, scale=float(SCALE),
                                )
                    # PV: accumulate over all 16 jt; lhsT = [vh | ones] (65 cols)
                    for hh in range(2):
                        h = 2 * pr + hh
                        for jt in range(NST):
                            nc.tensor.matmul(
                                pv_ps[hh][0:DK + 1, :],
                                vh_t[jt][:, h, :],
                                at_tiles[jt // 4][hh][:, jt % 4, :],
                                start=(jt == 0), stop=(jt == NST - 1),
                            )
                        # normalize: y = pv[0:64] * (1 / pv[64])
                        den = small.tile([1, 512], f32, tag="den")
                        nc.vector.tensor_copy(den[:], pv_ps[hh][DK:DK + 1, :])
                        rec = small.tile([1, 512], f32, tag="rec")
                        nc.vector.reciprocal_approx_fast(rec[:], den[:])
                        rb = small.tile([64, 512], f32, tag="rb")
                        nc.gpsimd.partition_broadcast(rb[:], rec[:])
                        nc.vector.tensor_tensor(
                            y_t[ft][ib][hh * 64:hh * 64 + 64, :],
                            pv_ps[hh][0:DK, :],
                            rb[:],
                            mult,
                        )

            # ---- output projection: out^T[e, s] = Wo-shard.T @ y ----
            for et in range(NDT):
                for sbh in range(2):
                    po = ps.tile([128, 2, 512], f32, tag="sc", bufs=3, name=f"po{et}{sbh}")
                    for j in range(2):
                        sb = sbh * 2 + j
                        for ft in range(2):
                            nc.tensor.matmul(
                                po[:, j, :],
                                wo_sb[:, ft, et * 128:(et + 1) * 128],
                                y_t[ft][sb][:],
                                start=(ft == 0), stop=(ft == 1),
                            )
                    o_sb = ostage.tile([128, 2, 512], f32, tag="ost")
                    nc.vector.tensor_copy(o_sb[:], po[:])
                    nc.sync.dma_start(
                        out.ap()[et * 128:(et + 1) * 128, sbh * 1024:(sbh + 1) * 1024],
                        o_sb[:].rearrange("p a b -> p (a b)"),
                    )

    nc.compile()
    return nc


def _get_nc():
    if "nc" not in _compiled:
        _compiled["nc"] = _build()
    return _compiled["nc"]


def kernel(q, k, v, Wq, bq, Wk, bk, Wv, bv, Wo, bo):
    outp, _ = _run(q, k, v, Wq, bq, Wk, bk, Wv, bv, Wo, bo)
    return outp


def _run(q, k, v, Wq, bq, Wk, bk, Wv, bv, Wo, bo, **run_kwargs):
    from concourse.bass_utils import run_bass_kernel_spmd

    nc = _get_nc()

    q = np.asarray(q, np.float32)
    k = np.asarray(k, np.float32)
    v = np.asarray(v, np.float32)
    Wq = np.asarray(Wq, np.float32)
    Wk = np.asarray(Wk, np.float32)
    Wv = np.asarray(Wv, np.float32)
    Wo = np.asarray(Wo, np.float32)
    bq = np.asarray(bq, np.float32)
    bk = np.asarray(bk, np.float32)
    bv = np.asarray(bv, np.float32)
    bo = np.asarray(bo, np.float32)

    import ml_dtypes
    bf = ml_dtypes.bfloat16
    xqT = [np.ascontiguousarray(q[b].T).astype(bf) for b in range(B)]
    xkT = [np.ascontiguousarray(k[b].T).astype(bf) for b in range(B)]
    xvT = [np.ascontiguousarray(v[b].T).astype(bf) for b in range(B)]

    in_maps = []
    for c in range(NCORES):
        b, hg = divmod(c, HG)
        rows = slice(hg * F, (hg + 1) * F)
        in_maps.append({
            "xq": xqT[b], "xk": xkT[b], "xv": xvT[b],
            "wq": np.ascontiguousarray(Wq[rows].T).astype(bf),
            "wk": np.ascontiguousarray(Wk[rows].T).astype(bf),
            "wv": np.ascontiguousarray(Wv[rows].T).astype(bf),
            "wo": np.ascontiguousarray(Wo[:, rows].T).astype(bf),
            "bq": np.ascontiguousarray(bq[rows].reshape(2, 128).T),
            "bk": np.ascontiguousarray(bk[rows].reshape(2, 128).T),
            "bv": np.ascontiguousarray(bv[rows].reshape(1, F)),
        })

    res = run_bass_kernel_spmd(nc, in_maps, core_ids=list(range(NCORES)), **run_kwargs)

    outp = np.empty((B, S, D), np.float32)
    for b in range(B):
        acc = res.results[b * HG]["out"].astype(np.float32)
        for hg in range(1, HG):
            acc = acc + res.results[b * HG + hg]["out"]
        outp[b] = acc.T + bo[None, :]
    return outp, res


# revision 12
# speedup vs baseline: 1.4139x; 1.1905x over previous
"""Multi-head self-attention (B=2, S=2048, D=1024, H=16) on 8 TRN2 NeuronCores.

Sharding: data-parallel over batch (2) x tensor-parallel over head-groups (4).
Core c = b*4 + hg handles batch b, heads hg*4..hg*4+3 (4 heads, 256 features).

Per-core device program (SPMD, identical on all cores):
  - QKV projections for the core's 256 output features (column-parallel)
  - full S x S attention for its 4 heads (softmax without max-subtraction,
    denominators via an appended ones-column in the PV matmul)
  - partial output projection (row-parallel): out_partial^T [1024, 2048]
Host: shards/transposes inputs, sums the 4 partial outputs per batch
(the "all-reduce"), adds bo, and untransposes.

All matmuls run in float32r (TF32-like, ~11 mantissa bits, 1 cycle/row on the
PE vs 4 for plain fp32); accumulation is fp32 in PSUM.
"""

import numpy as np

B, S, D = 2, 2048, 1024
H, DK = 16, 64
NCORES = 8
HG = 4          # head groups (tensor parallel)
HPG = 4         # heads per group
F = HPG * DK    # 256 local features per core
SCALE = 1.0 / np.sqrt(DK)

_compiled = {}


def _build():
    import concourse.bacc as bacc
    import concourse.tile as tile
    from concourse import mybir

    f32 = mybir.dt.float32
    bf16 = mybir.dt.bfloat16
    Exp = mybir.ActivationFunctionType.Exp
    mult = mybir.AluOpType.mult

    nc = bacc.Bacc("TRN2", target_bir_lowering=False, debug=False,
                   enable_asserts=True, num_devices=NCORES)

    xq = nc.dram_tensor("xq", (D, S), bf16, kind="ExternalInput")   # q[b].T
    xk = nc.dram_tensor("xk", (D, S), bf16, kind="ExternalInput")
    xv = nc.dram_tensor("xv", (D, S), bf16, kind="ExternalInput")
    wq = nc.dram_tensor("wq", (D, F), bf16, kind="ExternalInput")   # Wq[rows].T
    wk = nc.dram_tensor("wk", (D, F), bf16, kind="ExternalInput")
    wv = nc.dram_tensor("wv", (D, F), bf16, kind="ExternalInput")
    wo = nc.dram_tensor("wo", (F, D), bf16, kind="ExternalInput")   # Wo[:, cols].T
    bq = nc.dram_tensor("bq", (128, 2), f32, kind="ExternalInput")  # bias, f-tiled
    bk = nc.dram_tensor("bk", (128, 2), f32, kind="ExternalInput")
    bv = nc.dram_tensor("bv", (1, F), f32, kind="ExternalInput")
    out = nc.dram_tensor("out", (D, S), f32, kind="ExternalOutput")  # partial^T

    NDT = D // 128   # 8 d-tiles
    NST = S // 128   # 16 s-tiles (j tiles)
    NSB = S // 512   # 4 s-blocks (i blocks)

    with tile.TileContext(nc) as tc:
        import contextlib
        with contextlib.ExitStack() as ctx:
            consts = ctx.enter_context(tc.tile_pool(name="consts", bufs=1))
            big = ctx.enter_context(tc.tile_pool(name="big", bufs=28))
            acts = ctx.enter_context(tc.tile_pool(name="acts", bufs=1))
            ostage = ctx.enter_context(tc.tile_pool(name="ostage", bufs=3))
            small = ctx.enter_context(tc.tile_pool(name="small", bufs=2))
            ps = ctx.enter_context(tc.tile_pool(name="ps", bufs=1, space="PSUM"))

            # ---- constants ----
            wq_sb = consts.tile([128, NDT, F], bf16, tag="wq")
            wk_sb = consts.tile([128, NDT, F], bf16, tag="wk")
            wv_sb = consts.tile([128, NDT, F], bf16, tag="wv")
            nc.sync.dma_start(wq_sb[:], wq.ap().rearrange("(t p) f -> p t f", p=128))
            nc.sync.dma_start(wk_sb[:], wk.ap().rearrange("(t p) f -> p t f", p=128))
            nc.sync.dma_start(wv_sb[:], wv.ap().rearrange("(t p) f -> p t f", p=128))
            wo_sb = consts.tile([128, 2, D], bf16, tag="wo")
            nc.sync.dma_start(wo_sb[:], wo.ap().rearrange("(t p) e -> p t e", p=128))
            bq_sb = consts.tile([128, 2], f32, tag="bq")
            bk_sb = consts.tile([128, 2], f32, tag="bk")
            nc.sync.dma_start(bq_sb[:], bq.ap()[:])
            nc.sync.dma_start(bk_sb[:], bk.ap()[:])
            bv_sb = consts.tile([128, F], f32, tag="bv")
            nc.sync.dma_start(bv_sb[:], bv.ap().to_broadcast((128, F)))

            # persistent activations
            # qh/kh: [f, s] transposed projections, per (ft, sb) tiles
            qh_t = [[acts.tile([128, 512], bf16, tag=f"qh{ft}{sb}", name=f"qh{ft}{sb}")
                     for sb in range(NSB)] for ft in range(2)]
            kh_t = [[acts.tile([128, 512], bf16, tag=f"kh{ft}{sb}", name=f"kh{ft}{sb}")
                     for sb in range(NSB)] for ft in range(2)]
            # vh: [s, h, c+1] with ones column at c=64 (PV denominator trick)
            vh_t = [acts.tile([128, HPG, DK + 1], bf16, tag=f"vh{st}", name=f"vh{st}")
                    for st in range(NST)]
            for st in range(NST):
                nc.vector.memset(vh_t[st][:, :, DK:DK + 1], 1.0)
            # y: normalized attention output, [f, s] per (ft, sb)
            y_t = [[acts.tile([128, 512], bf16, tag=f"y{ft}{sb}", name=f"y{ft}{sb}")
                    for sb in range(NSB)] for ft in range(2)]

            # ---- input DMAs (xq, xk, xv streams; tiles stay resident) ----
            def load_x(xdram, pfx):
                ts = []
                for dt in range(NDT):
                    t = big.tile([128, S], bf16, tag="big", name=f"{pfx}{dt}")
                    nc.sync.dma_start(t[:], xdram.ap()[dt * 128:(dt + 1) * 128, :])
                    ts.append(t)
                return ts

            xqt = load_x(xq, "xq")
            xkt = load_x(xk, "xk")
            xvt = load_x(xv, "xv")

            # ---- projection pass helpers (per (ft, sb) single-bank accum) ----
            def qk_pass(w_sb, b_sb, xts, dst, ft, pfx):
                for sb in range(NSB):
                    p = ps.tile([128, 512], f32, tag="w1", bufs=4, name=f"{pfx}{ft}{sb}")
                    for dt in range(NDT):
                        nc.tensor.matmul(
                            p[:],
                            w_sb[:, dt, ft * 128:(ft + 1) * 128],
                            xts[dt][:, sb * 512:(sb + 1) * 512],
                            start=(dt == 0), stop=(dt == NDT - 1),
                        )
                    nc.vector.tensor_scalar_add(dst[ft][sb][:], p[:], b_sb[:, ft:ft + 1])

            def v_pass():
                for st in range(NST):
                    p = ps.tile([128, 512], f32, tag="w1", bufs=4, name=f"vps{st}")
                    for dt in range(NDT):
                        nc.tensor.matmul(
                            p[:, 0:F],
                            xvt[dt][:, st * 128:(st + 1) * 128],
                            wv_sb[:, dt, :],
                            start=(dt == 0), stop=(dt == NDT - 1),
                        )
                    nc.vector.tensor_tensor(
                        vh_t[st][:, :, 0:DK],
                        p[:, 0:F].rearrange("p (h c) -> p h c", h=HPG),
                        bv_sb[:].rearrange("p (h c) -> p h c", h=HPG),
                        mybir.AluOpType.add,
                    )

            # ---- attention for one head pair (row-packed QK, PV + ones col) ----
            def attention_pair(pr):
                ft = pr
                for ib in range(NSB):
                    pv_ps = [ps.tile([128, 512], f32, tag="w1", bufs=4,
                                     name=f"pv{pr}{ib}_{i}") for i in range(2)]
                    at_tiles = []
                    for jc in range(NST // 4):
                        at = [big.tile([128, 4, 512], bf16, tag="big",
                                       name=f"at{pr}{ib}{jc}_{i}") for i in range(2)]
                        at_tiles.append(at)
                        for half in range(2):
                            sc = [None, None]
                            for hh in range(2):
                                base = hh * 64
                                sc[hh] = ps.tile([128, 2, 512], f32, tag="w2",
                                                 bufs=2, name=f"sc{hh}")
                                for jj in range(2):
                                    jt = jc * 4 + half * 2 + jj
                                    nc.tensor.matmul(
                                        sc[hh][:, jj, :],
                                        kh_t[ft][jt // 4][base:base + 64,
                                                          (jt % 4) * 128:(jt % 4 + 1) * 128],
                                        qh_t[ft][ib][base:base + 64, :],
                                        start=True, stop=True,
                                        tile_position=(base, 0),
                                    )
                            for hh in range(2):
                                nc.scalar.activation(
                                    at[hh][:, half * 2:half * 2 + 2, :],
                                    sc[hh][:, :, :],
                                    Exp, scale=float(SCALE),
                                )
                    for hh in range(2):
                        h = 2 * pr + hh
                        for jt in range(NST):
                            nc.tensor.matmul(
                                pv_ps[hh][0:DK + 1, :],
                                vh_t[jt][:, h, :],
                                at_tiles[jt // 4][hh][:, jt % 4, :],
                                start=(jt == 0), stop=(jt == NST - 1),
                            )
                        den = small.tile([1, 512], f32, tag="den")
                        nc.vector.tensor_copy(den[:], pv_ps[hh][DK:DK + 1, :])
                        rec = small.tile([1, 512], f32, tag="rec")
                        nc.vector.reciprocal_approx_fast(rec[:], den[:])
                        rb = small.tile([64, 512], f32, tag="rb")
                        nc.gpsimd.partition_broadcast(rb[:], rec[:])
                        nc.vector.tensor_tensor(
                            y_t[ft][ib][hh * 64:hh * 64 + 64, :],
                            pv_ps[hh][0:DK, :],
                            rb[:],
                            mult,
                        )

            # ---- phase schedule: start pair-0 attention early; ft=1
            # projections fill PE gaps during the ACT-paced attention ----
            qk_pass(wq_sb, bq_sb, xqt, qh_t, 0, "psq")
            qk_pass(wk_sb, bk_sb, xkt, kh_t, 0, "psk")
            v_pass()
            attention_pair(0)
            qk_pass(wq_sb, bq_sb, xqt, qh_t, 1, "psq")
            qk_pass(wk_sb, bk_sb, xkt, kh_t, 1, "psk")
            attention_pair(1)

            # ---- output projection: out^T[e, s] = Wo-shard.T @ y ----
            for et in range(NDT):
                for sbh in range(2):
                    po = ps.tile([128, 2, 512], f32, tag="w2", bufs=2, name=f"po{et}{sbh}")
                    for j in range(2):
                        sb = sbh * 2 + j
                        for ft in range(2):
                            nc.tensor.matmul(
                                po[:, j, :],
                                wo_sb[:, ft, et * 128:(et + 1) * 128],
                                y_t[ft][sb][:],
                                start=(ft == 0), stop=(ft == 1),
                            )
                    o_sb = ostage.tile([128, 2, 512], f32, tag="ost")
                    nc.vector.tensor_copy(o_sb[:], po[:])
                    nc.sync.dma_start(
                        out.ap()[et * 128:(et + 1) * 128, sbh * 1024:(sbh + 1) * 1024],
                        o_sb[:].rearrange("p a b -> p (a b)"),
                    )

    nc.compile()
    return nc


def _get_nc():
    if "nc" not in _compiled:
        _compiled["nc"] = _build()
    return _compiled["nc"]


def kernel(q, k, v, Wq, bq, Wk, bk, Wv, bv, Wo, bo):
    outp, _ = _run(q, k, v, Wq, bq, Wk, bk, Wv, bv, Wo, bo)
    return outp


def _run(q, k, v, Wq, bq, Wk, bk, Wv, bv, Wo, bo, **run_kwargs):
    from concourse.bass_utils import run_bass_kernel_spmd

    nc = _get_nc()

    q = np.asarray(q, np.float32)
    k = np.asarray(k, np.float32)
    v = np.asarray(v, np.float32)
    Wq = np.asarray(Wq, np.float32)
    Wk = np.asarray(Wk, np.float32)
    Wv = np.asarray(Wv, np.float32)
    Wo = np.asarray(Wo, np.float32)
    bq = np.asarray(bq, np.float32)
    bk = np.asarray(bk, np.float32)
    bv = np.asarray(bv, np.float32)
    bo = np.asarray(bo, np.float32)

    import ml_dtypes
    bf = ml_dtypes.bfloat16
    xqT = [np.ascontiguousarray(q[b].T).astype(bf) for b in range(B)]
    xkT = [np.ascontiguousarray(k[b].T).astype(bf) for b in range(B)]
    xvT = [np.ascontiguousarray(v[b].T).astype(bf) for b in range(B)]

    in_maps = []
    for c in range(NCORES):
        b, hg = divmod(c, HG)
        rows = slice(hg * F, (hg + 1) * F)
        in_maps.append({
            "xq": xqT[b], "xk": xkT[b], "xv": xvT[b],
            "wq": np.ascontiguousarray(Wq[rows].T).astype(bf),
            "wk": np.ascontiguousarray(Wk[rows].T).astype(bf),
            "wv": np.ascontiguousarray(Wv[rows].T).astype(bf),
            "wo": np.ascontiguousarray(Wo[:, rows].T).astype(bf),
            "bq": np.ascontiguousarray(bq[rows].reshape(2, 128).T),
            "bk": np.ascontiguousarray(bk[rows].reshape(2, 128).T),
            "bv": np.ascontiguousarray(bv[rows].reshape(1, F)),
        })

    res = run_bass_kernel_spmd(nc, in_maps, core_ids=list(range(NCORES)), **run_kwargs)

    outp = np.empty((B, S, D), np.float32)
    for b in range(B):
        acc = res.results[b * HG]["out"].astype(np.float32)
        for hg in range(1, HG):
            acc = acc + res.results[b * HG + hg]["out"]
        outp[b] = acc.T + bo[None, :]
    return outp, res
